# revision 1
# baseline (speedup 1.0000x reference)
"""Trainium2 Bass kernel for DeformableCrossAttentionModule.

Math (per batch b):
  offset = conv3x3(query, w_off) + b_off            # (18, H, W); ch 0:9 = dy, 9:18 = dx
  mod    = sigmoid(conv3x3(query, w_mod) + b_mod)   # (9, H, W)
  py/px  = base grid + kernel offset + offset       # (9, H, W)
  samp   = bilinear_sample(pad(value), px, py)      # (C, H, W, 9), zeros padding
  out    = einsum('chwn,ocn->bohw', samp * mod, w_out)

Sharding: 8 cores = (batch b in 0..3) x (row-half in 0..1); each core handles
32 output rows, streamed as 16 chunks of 128 positions (2 rows).

The axon-tunneled runtime rejects dynamic-offset DMA (indirect_dma_start /
dma_gather execute-fail), so the kernel runs in two device phases with the
bilinear x-pair gather — pure data movement — done on host between them:
  phase 1 (device): conv3x3 on PE, sampling coords / corner weights /
      flat indices on DVE -> idx + corner-weight tensors in DRAM
  host: fancy-index gather of (y, x0..x0+1) channel pairs (fp16)
  phase 2 (device): bilinear blend via diagonal-matrix matmuls on PE
      (PSUM-accumulated, also transposes to channel-major), then the
      1x1xN output projection as accumulating matmuls (fp16 in, fp32 acc)
"""

import sys

for _p in ("/opt/trn_rl_repo", "/opt/pypackages"):
    if _p not in sys.path:
        sys.path.insert(0, _p)

from contextlib import ExitStack

import numpy as np

import concourse.bacc as bacc
import concourse.bass as bass
import concourse.tile as tile
from concourse import mybir
from concourse.bass_utils import run_bass_kernel_spmd

F32 = mybir.dt.float32
F16 = mybir.dt.float16
I32 = mybir.dt.int32

B, C, H, W = 4, 256, 64, 64
N, PAD, OUTC = 9, 1, 256
Hp, Wp = H + 2 * PAD, W + 2 * PAD  # 66, 66
NCORES = 8
ROWS = H // 2          # output rows per core = 32
NCHUNK = ROWS // 2     # 16 chunks of 128 positions (2 rows x 64 cols)
ASCALE = float(Wp) / float(Wp - 1)  # 66/65, same for y since Hp == Wp
BIAS = 16.0            # keeps coords positive so trunc == floor


def _build_phase1():
    nc = bacc.Bacc("TRN2", target_bir_lowering=False, debug=False,
                   num_devices=NCORES)

    qs_d = nc.dram_tensor("qs", (3, 2, 128, H // 2 + 2, W), F32,
                          kind="ExternalInput").ap()
    wc_d = nc.dram_tensor("wc", (128, 9 * 2 * 27), F32,
                          kind="ExternalInput").ap()
    yb_d = nc.dram_tensor("ybase", (128, NCHUNK * N), F32,
                          kind="ExternalInput").ap()
    xb_d = nc.dram_tensor("xbase", (128, NCHUNK * N), F32,
                          kind="ExternalInput").ap()
    mb_d = nc.dram_tensor("mbias", (128, NCHUNK * N), F32,
                          kind="ExternalInput").ap()
    idx_d = nc.dram_tensor("idxo", (128, NCHUNK * 18), I32,
                           kind="ExternalOutput").ap()
    w4_d = nc.dram_tensor("w4o", (128, NCHUNK * 36), F16,
                          kind="ExternalOutput").ap()

    mult = mybir.AluOpType.mult
    add = mybir.AluOpType.add
    sub = mybir.AluOpType.subtract
    opmax = mybir.AluOpType.max
    opmin = mybir.AluOpType.min
    iseq = mybir.AluOpType.is_equal

    with tile.TileContext(nc) as tc, ExitStack() as ctx:
        cpool = ctx.enter_context(tc.tile_pool(name="const", bufs=1))
        wkpool = ctx.enter_context(tc.tile_pool(name="work", bufs=3))
        pcv = ctx.enter_context(tc.tile_pool(name="pconv", bufs=4,
                                             space="PSUM"))

        qtiles = {}
        for dx in range(3):
            for blk in range(2):
                qt = cpool.tile([128, 34 * W], F32, name=f"qs{dx}{blk}",
                                tag=f"qs{dx}{blk}")
                nc.sync.dma_start(qt[:], qs_d[dx, blk])
                qtiles[(dx, blk)] = qt
        wct = cpool.tile([128, 9 * 2 * 27], F32, tag="wc")
        nc.sync.dma_start(wct[:], wc_d[:])
        ybt = cpool.tile([128, NCHUNK * N], F32, tag="yb")
        nc.sync.dma_start(ybt[:], yb_d[:])
        xbt = cpool.tile([128, NCHUNK * N], F32, tag="xb")
        nc.sync.dma_start(xbt[:], xb_d[:])
        mbt = cpool.tile([128, NCHUNK * N], F32, tag="mb")
        nc.sync.dma_start(mbt[:], mb_d[:])

        for t in range(NCHUNK):
            pc = pcv.tile([128, 27], F32)
            for tap in range(9):
                dy, dx = divmod(tap, 3)
                for blk in range(2):
                    qo = (2 * t + dy) * W
                    lhsT = qtiles[(dx, blk)][:, qo: qo + 128]
                    co = (tap * 2 + blk) * 27
                    nc.tensor.matmul(
                        pc[:], lhsT=lhsT, rhs=wct[:, co: co + 27],
                        start=(tap == 0 and blk == 0),
                        stop=(tap == 8 and blk == 1),
                    )

            wk = wkpool.tile([128, 128], F32, tag="wk")

            def s(i):
                return wk[:, 9 * i: 9 * i + 9]

            cb9 = t * N
            oy, ox, ml = pc[:, 0:9], pc[:, 9:18], pc[:, 18:27]
            v = nc.vector
            v.scalar_tensor_tensor(s(0), oy, ASCALE, ybt[:, cb9: cb9 + 9],
                                   op0=mult, op1=add)
            v.scalar_tensor_tensor(s(1), ox, ASCALE, xbt[:, cb9: cb9 + 9],
                                   op0=mult, op1=add)
            v.tensor_tensor(s(13), ml, mbt[:, cb9: cb9 + 9], op=add)
            nc.scalar.activation(s(12), s(13),
                                 mybir.ActivationFunctionType.Sigmoid)
            # floor(y) robust to the cast rounding mode (trunc on sim, RNE
            # on hw): c = int(y); y0 = c - (c > y)
            flr = wkpool.tile([128, 18], I32, tag="flr")
            v.tensor_copy(out=flr[:, 0:9], in_=s(0))
            v.tensor_copy(out=flr[:, 9:18], in_=s(1))
            v.tensor_copy(out=s(4), in_=flr[:, 0:9])
            v.tensor_copy(out=s(5), in_=flr[:, 9:18])
            v.tensor_tensor(s(2), s(4), s(0), op=mybir.AluOpType.is_gt)
            v.tensor_tensor(s(3), s(5), s(1), op=mybir.AluOpType.is_gt)
            v.tensor_tensor(s(4), s(4), s(2), op=sub)        # y0 = floor
            v.tensor_tensor(s(5), s(5), s(3), op=sub)        # x0 = floor
            v.tensor_tensor(s(2), s(0), s(4), op=sub)        # fy
            v.tensor_tensor(s(3), s(1), s(5), op=sub)        # fx
            v.tensor_scalar(s(6), s(4), BIAS, BIAS + 64.0, op0=opmax,
                            op1=opmin)                        # y0c
            v.tensor_scalar(s(7), s(5), BIAS, BIAS + 64.0, op0=opmax,
                            op1=opmin)                        # x0c
            # row A = pixel y0c, row B = y0c+1; with d = y0c - y0:
            #   wA = [d==0]*(1-f) + [d==1]*f ;  wB = [d==0]*f + [d==-1]*(1-f)
            v.tensor_tensor(s(8), s(6), s(4), op=sub)         # d_y
            v.tensor_scalar(s(4), s(8), 0.0, None, op0=iseq)  # e0y
            v.tensor_scalar(s(10), s(8), 1.0, None, op0=iseq)   # e1y
            v.tensor_scalar(s(8), s(8), -1.0, None, op0=iseq)   # em1y
            v.tensor_scalar(s(13), s(2), -1.0, 1.0, op0=mult, op1=add)
            v.tensor_tensor(s(11), s(4), s(13), op=mult)
            v.tensor_tensor(s(10), s(10), s(2), op=mult)
            v.tensor_tensor(s(10), s(11), s(10), op=add)      # wyA
            v.tensor_tensor(s(11), s(4), s(2), op=mult)
            v.tensor_tensor(s(8), s(8), s(13), op=mult)
            v.tensor_tensor(s(2), s(11), s(8), op=add)        # wyB
            v.tensor_tensor(s(10), s(10), s(12), op=mult)     # wyA * mod
            v.tensor_tensor(s(2), s(2), s(12), op=mult)       # wyB * mod

            v.tensor_tensor(s(9), s(7), s(5), op=sub)         # d_x
            v.tensor_scalar(s(5), s(9), 0.0, None, op0=iseq)  # e0x
            v.tensor_scalar(s(11), s(9), 1.0, None, op0=iseq)   # e1x
            v.tensor_scalar(s(9), s(9), -1.0, None, op0=iseq)   # em1x
            v.tensor_scalar(s(13), s(3), -1.0, 1.0, op0=mult, op1=add)
            v.tensor_tensor(s(4), s(5), s(13), op=mult)
            v.tensor_tensor(s(11), s(11), s(3), op=mult)
            v.tensor_tensor(s(11), s(4), s(11), op=add)       # wxA
            v.tensor_tensor(s(4), s(5), s(3), op=mult)
            v.tensor_tensor(s(9), s(9), s(13), op=mult)
            v.tensor_tensor(s(3), s(4), s(9), op=add)         # wxB

            w4 = wkpool.tile([128, 36], F16, tag="w4")
            v.tensor_tensor(w4[:, 0:9], s(10), s(11), op=mult)    # A,pixA
            v.tensor_tensor(w4[:, 9:18], s(10), s(3), op=mult)    # A,pixB
            v.tensor_tensor(w4[:, 18:27], s(2), s(11), op=mult)   # B,pixA
            v.tensor_tensor(w4[:, 27:36], s(2), s(3), op=mult)    # B,pixB

            # flat gather indices: idx = (y0c-16)*66 + (x0c-16); row B = +66
            v.scalar_tensor_tensor(s(0), s(6), 66.0, s(7), op0=mult, op1=add)
            v.tensor_scalar(s(1), s(0), -(BIAS * 66.0 + BIAS), None, op0=add)
            v.tensor_scalar(s(3), s(1), 66.0, None, op0=add)
            idx32 = wkpool.tile([128, 18], I32, tag="idx")
            v.tensor_copy(out=idx32[:, 0:9], in_=s(1))
            v.tensor_copy(out=idx32[:, 9:18], in_=s(3))

            nc.sync.dma_start(idx_d[:, t * 18: (t + 1) * 18], idx32[:])
            nc.sync.dma_start(w4_d[:, t * 36: (t + 1) * 36], w4[:])

    nc.compile()
    return nc


def _build_phase2():
    nc = bacc.Bacc("TRN2", target_bir_lowering=False, debug=False,
                   num_devices=NCORES)

    g_d = nc.dram_tensor("gath", (NCHUNK, 128, 18 * 512), F16,
                         kind="ExternalInput").ap()
    w4_d = nc.dram_tensor("w4o", (128, NCHUNK * 36), F16,
                          kind="ExternalInput").ap()
    w2_d = nc.dram_tensor("w2", (128, N * 2 * 2 * 128), F16,
                          kind="ExternalInput").ap()
    id_d = nc.dram_tensor("ident", (128, 128), F16,
                          kind="ExternalInput").ap()
    out_d = nc.dram_tensor("out", (OUTC, ROWS, W), F32,
                           kind="ExternalOutput").ap()

    mult = mybir.AluOpType.mult

    with tile.TileContext(nc) as tc, ExitStack() as ctx:
        cpool = ctx.enter_context(tc.tile_pool(name="const", bufs=1))
        gpool = ctx.enter_context(tc.tile_pool(name="gath", bufs=3))
        dpool = ctx.enter_context(tc.tile_pool(name="diag", bufs=2))
        spool = ctx.enter_context(tc.tile_pool(name="samp", bufs=3))
        opool = ctx.enter_context(tc.tile_pool(name="ostg", bufs=2))
        psm = ctx.enter_context(tc.tile_pool(name="psamp", bufs=4,
                                             space="PSUM"))
        pout = ctx.enter_context(tc.tile_pool(name="pout", bufs=2,
                                              space="PSUM"))

        w2t = cpool.tile([128, N * 2 * 2 * 128], F16, tag="w2")
        nc.sync.dma_start(w2t[:], w2_d[:])
        w4t = cpool.tile([128, NCHUNK * 36], F16, tag="w4t")
        nc.sync.dma_start(w4t[:], w4_d[:])
        idt = cpool.tile([128, 128], F16, tag="id")
        nc.sync.dma_start(idt[:], id_d[:])

        for t in range(NCHUNK):
            gt = gpool.tile([128, 18 * 512], F16, tag="gt")
            nc.sync.dma_start(gt[:], g_d[t])
            gv = gt[:].rearrange("p (s e) -> p s e", e=512)

            # diag bank: bank[p, (k*9+n)*128 + f] = I[p, f] * w4[p, k*9+n]
            bank = dpool.tile([128, 36 * 128], F16, tag="bank")
            nc.vector.tensor_tensor(
                out=bank[:].rearrange("p (s f) -> p s f", f=128),
                in0=idt[:].rearrange("p (u f) -> p u f", u=1).to_broadcast(
                    [128, 36, 128]),
                in1=w4t[:, t * 36: (t + 1) * 36].rearrange(
                    "p (s u) -> p s u", u=1).to_broadcast([128, 36, 128]),
                op=mult,
            )

            po = [pout.tile([128, 128], F32, name=f"po{ob}", tag=f"po{ob}")
                  for ob in range(2)]
            for n in range(N):
                ps = psm.tile([128, 256], F32, tag="ps")
                for cb in range(2):
                    for r in range(2):
                        for pix in range(2):
                            k = r * 2 + pix
                            lo = pix * 256 + cb * 128
                            bo = (k * 9 + n) * 128
                            nc.tensor.matmul(
                                ps[:, cb * 128: cb * 128 + 128],
                                lhsT=gv[:, r * 9 + n, lo: lo + 128],
                                rhs=bank[:, bo: bo + 128],
                                start=(k == 0), stop=(k == 3),
                            )
                sampn = spool.tile([128, 256], F16, tag="sampn")
                nc.scalar.copy(sampn[:], ps[:])
                for cb in range(2):
                    for ob in range(2):
                        wo = ((n * 2 + cb) * 2 + ob) * 128
                        nc.tensor.matmul(
                            po[ob][:],
                            lhsT=w2t[:, wo: wo + 128],
                            rhs=sampn[:, cb * 128: cb * 128 + 128],
                            start=(n == 0 and cb == 0),
                            stop=(n == 8 and cb == 1),
                        )

            ost = opool.tile([128, 256], F32, tag="ost")
            nc.scalar.copy(ost[:, 0:128], po[0][:])
            nc.scalar.copy(ost[:, 128:256], po[1][:])
            for ob in range(2):
                nc.sync.dma_start(
                    out=out_d[ob * 128: ob * 128 + 128, 2 * t: 2 * t + 2, :],
                    in_=ost[:, ob * 128: ob * 128 + 128],
                )

    nc.compile()
    return nc


_CACHE = {}


def _get_programs():
    if "p1" not in _CACHE:
        _CACHE["p1"] = _build_phase1()
        _CACHE["p2"] = _build_phase2()
    return _CACHE["p1"], _CACHE["p2"]


def _host_prep(query, value, w_off, b_off, w_mod, b_mod, w_out):
    query = np.asarray(query, dtype=np.float32)
    value = np.asarray(value, dtype=np.float32)
    w_off = np.asarray(w_off, dtype=np.float32)
    b_off = np.asarray(b_off, dtype=np.float32)
    w_mod = np.asarray(w_mod, dtype=np.float32)
    b_mod = np.asarray(b_mod, dtype=np.float32)
    w_out = np.asarray(w_out, dtype=np.float32)

    qp = np.zeros((B, 2, 128, Hp, Wp), np.float32)
    qp[:, :, :, PAD:PAD + H, PAD:PAD + W] = query.reshape(B, 2, 128, H, W)
    qsx = np.stack([qp[:, :, :, :, dx: dx + W] for dx in range(3)], axis=1)

    vp = np.zeros((B, C, Hp, Wp), np.float32)
    vp[:, :, PAD:PAD + H, PAD:PAD + W] = value
    vcl = np.ascontiguousarray(
        vp.transpose(0, 2, 3, 1).reshape(B, Hp * Wp * C)).astype(np.float16)

    w27 = np.concatenate([w_off, w_mod], axis=0)
    wc = np.ascontiguousarray(
        w27.reshape(27, 2, 128, 9).transpose(2, 3, 1, 0)
    ).reshape(128, 9 * 2 * 27).astype(np.float32)

    w2 = np.ascontiguousarray(
        w_out.reshape(2, 128, 2, 128, N).transpose(3, 4, 2, 0, 1)
    ).reshape(128, N * 2 * 2 * 128).astype(np.float16)

    ident = np.eye(128, dtype=np.float16)

    n_ar = np.arange(N)
    pn_r = (n_ar // 3 - 1).astype(np.float32)
    pn_c = (n_ar % 3 - 1).astype(np.float32)
    p_ar = np.arange(128)
    row_in_chunk = (p_ar // W).astype(np.float32)
    col_in_chunk = (p_ar % W).astype(np.float32)
    t_ar = np.arange(NCHUNK, dtype=np.float32)

    xb = (ASCALE * (col_in_chunk[:, None, None] + pn_c[None, None, :]
                    + b_off[N:2 * N][None, None, :]) - 0.5 + BIAS)
    xb = np.broadcast_to(xb, (128, NCHUNK, N)).reshape(128, NCHUNK * N)
    xb = np.ascontiguousarray(xb, dtype=np.float32)
    mb = np.broadcast_to(b_mod[None, None, :], (128, NCHUNK, N))
    mb = np.ascontiguousarray(mb.reshape(128, NCHUNK * N), dtype=np.float32)

    in1, in2 = [], []
    for core in range(NCORES):
        b, half = divmod(core, 2)
        r0 = half * ROWS
        yb = (ASCALE * (r0 + 2.0 * t_ar[None, :, None]
                        + row_in_chunk[:, None, None] + pn_r[None, None, :]
                        + b_off[0:N][None, None, :]) - 0.5 + BIAS)
        yb = np.ascontiguousarray(
            yb.reshape(128, NCHUNK * N), dtype=np.float32)
        in1.append({
            "qs": np.ascontiguousarray(qsx[b, :, :, :, r0: r0 + 34, :]),
            "wc": wc,
            "ybase": yb,
            "xbase": xb,
            "mbias": mb,
        })
        in2.append({
            "w4o": None,  # filled after phase 1
            "gath": None,
            "w2": w2,
            "ident": ident,
        })
    return in1, in2, vcl


def kernel(**inputs):
    p1, p2 = _get_programs()
    in1, in2, vcl = _host_prep(**inputs)

    res1 = run_bass_kernel_spmd(p1, in1, core_ids=list(range(NCORES)))

    # host gather of bilinear x-pairs (pure data movement)
    off = np.arange(512)
    for core in range(NCORES):
        b = core // 2
        idx = res1.results[core]["idxo"].reshape(128, NCHUNK, 18)
        # gath[t, p, s*512:(s+1)*512] = vcl[b][idx[p,t,s]*256 : +512]
        gidx = (idx.transpose(1, 0, 2).reshape(NCHUNK, 128, 18, 1) * 256
                + off).reshape(NCHUNK, 128, 18 * 512)
        in2[core]["gath"] = vcl[b][gidx]
        in2[core]["w4o"] = res1.results[core]["w4o"]

    res2 = run_bass_kernel_spmd(p2, in2, core_ids=list(range(NCORES)))

    out = np.empty((B, OUTC, H, W), np.float32)
    for core in range(NCORES):
        b, half = divmod(core, 2)
        r0 = half * ROWS
        out[b, :, r0: r0 + ROWS, :] = res2.results[core]["out"]
    return out



# revision 2
# speedup vs baseline: 6.4657x; 6.4657x over previous
"""Trainium2 Bass kernel for DeformableCrossAttentionModule (single phase).

Math (per batch b):
  offset = conv3x3(query, w_off) + b_off            # (18, H, W); ch 0:9 = dy, 9:18 = dx
  mod    = sigmoid(conv3x3(query, w_mod) + b_mod)   # (9, H, W)
  py/px  = base grid + kernel offset + offset       # (9, H, W)
  samp   = bilinear_sample(pad(value), px, py)      # (C, H, W, 9), zeros padding
  out    = einsum('chwn,ocn->bohw', samp * mod, w_out)

Sharding: 8 cores = (batch b in 0..3) x (row-half in 0..1); each core handles
32 output rows, streamed as 16 chunks of 128 positions (2 rows).

Everything runs in ONE device program per core:
  conv3x3 on PE -> sampling coords / corner weights / pixel indices on DVE
  -> bilinear 4-corner gather of channel pairs from the padded value image
     held in SBUF via the GPSIMD ap_gather instruction (per-16-partition
     wrapped index lists, rearranged on-device with 15 small SB->SB DMAs)
  -> output projection as accumulating PE matmuls (gathered data is
     channel-major so the projection needs no transpose); the per-position
     bilinear-corner weight x modulator is applied as a per-partition scale
     on the ACT engine between the projection matmul (pos-major PSUM) and an
     identity-matmul accumulation over the 36 (corner, tap) terms.

The per-core output is (2048, 256) f16 pos-major; the host transposes to
channel-major fp32 during reassembly.
"""

import sys

for _p in ("/opt/trn_rl_repo", "/opt/pypackages"):
    if _p not in sys.path:
        sys.path.insert(0, _p)

from contextlib import ExitStack

import numpy as np

import concourse.bacc as bacc
import concourse.tile as tile
from concourse import mybir
from concourse.bass_utils import run_bass_kernel_spmd

F32 = mybir.dt.float32
F16 = mybir.dt.float16
I32 = mybir.dt.int32
I16 = mybir.dt.int16

B, C, H, W = 4, 256, 64, 64
N, PAD, OUTC = 9, 1, 256
Hp, Wp = H + 2 * PAD, W + 2 * PAD  # 66, 66
NPIX = Hp * Wp                     # 4356
NCORES = 8
ROWS = H // 2          # output rows per core = 32
NCHUNK = ROWS // 2     # 16 chunks of 128 positions (2 rows x 64 cols)
TN = NCHUNK * N        # 144 = (chunk, tap) coordinate columns
ASCALE = float(Wp) / float(Wp - 1)  # 66/65, same for y since Hp == Wp
BIAS = 16.0            # keeps coords positive so trunc == floor


def _build():
    nc = bacc.Bacc("TRN2", target_bir_lowering=False, debug=False,
                   num_devices=NCORES)

    qp_d = nc.dram_tensor("qp", (2, 128, 34 * Wp), F16,
                          kind="ExternalInput").ap()
    wc_d = nc.dram_tensor("wc", (128, 9 * 2 * 27), F16,
                          kind="ExternalInput").ap()
    vv_d = nc.dram_tensor("vv", (128, NPIX * 2), F16,
                          kind="ExternalInput").ap()
    w2_d = nc.dram_tensor("w2", (128, 2 * N * 256), F16,
                          kind="ExternalInput").ap()
    yb_d = nc.dram_tensor("ybase", (128, TN), F32, kind="ExternalInput").ap()
    xb_d = nc.dram_tensor("xbase", (128, TN), F32, kind="ExternalInput").ap()
    mb_d = nc.dram_tensor("mbias", (128, TN), F32, kind="ExternalInput").ap()
    id_d = nc.dram_tensor("ident", (128, 128), F16, kind="ExternalInput").ap()
    out_d = nc.dram_tensor("out", (NCHUNK * 128, OUTC), F16,
                           kind="ExternalOutput").ap()

    mult = mybir.AluOpType.mult
    add = mybir.AluOpType.add
    sub = mybir.AluOpType.subtract
    opmax = mybir.AluOpType.max
    opmin = mybir.AluOpType.min
    iseq = mybir.AluOpType.is_equal
    isgt = mybir.AluOpType.is_gt
    v = nc.vector

    with tile.TileContext(nc) as tc, ExitStack() as ctx:
        cpool = ctx.enter_context(tc.tile_pool(name="const", bufs=1))
        wkpool = ctx.enter_context(tc.tile_pool(name="work", bufs=1))
        gpool = ctx.enter_context(tc.tile_pool(name="gath", bufs=2))
        dpool = ctx.enter_context(tc.tile_pool(name="deint", bufs=2))
        spool = ctx.enter_context(tc.tile_pool(name="sct", bufs=3))
        opool = ctx.enter_context(tc.tile_pool(name="ostg", bufs=2))

        # ---- load constants ----
        qpt = []
        for blk in range(2):
            qt = cpool.tile([128, 34 * Wp], F16, name=f"qp{blk}",
                            tag=f"qp{blk}")
            nc.sync.dma_start(qt[:], qp_d[blk])
            qpt.append(qt)
        wct = cpool.tile([128, 9 * 2 * 27], F16, tag="wc")
        nc.sync.dma_start(wct[:], wc_d[:])
        vvt = cpool.tile([128, NPIX * 2], F16, tag="vv")
        nc.sync.dma_start(vvt[:], vv_d[:])
        w2t = cpool.tile([128, 2 * N * 256], F16, tag="w2")
        nc.sync.dma_start(w2t[:], w2_d[:])
        ybt = cpool.tile([128, TN], F32, tag="yb")
        nc.sync.dma_start(ybt[:], yb_d[:])
        xbt = cpool.tile([128, TN], F32, tag="xb")
        nc.sync.dma_start(xbt[:], xb_d[:])
        mbt = cpool.tile([128, TN], F32, tag="mb")
        nc.sync.dma_start(mbt[:], mb_d[:])
        idt = cpool.tile([128, 128], F16, tag="id")
        nc.sync.dma_start(idt[:], id_d[:])

        # ---- shifted query copies (conv lhsT needs contiguous 128-wide
        # position windows, i.e. width-64 row layout per dx shift) ----
        qs = {}
        for dx in range(3):
            for blk in range(2):
                qt = cpool.tile([128, 34 * W], F16, name=f"qs{dx}{blk}",
                                tag=f"qs{dx}{blk}")
                src = qpt[blk][:].rearrange("p (h w) -> p h w", w=Wp)
                dst = qt[:].rearrange("p (h w) -> p h w", w=W)
                v.tensor_copy(out=dst, in_=src[:, :, dx: dx + W])
                qs[(dx, blk)] = qt

        # ---- stage A: conv3x3 for all chunks -> pcall [128, 16*27] ----
        pcall = wkpool.tile([128, NCHUNK * 27], F32, tag="pcall")
        with tc.tile_pool(name="pconv", bufs=2, space="PSUM") as pcv:
            for t in range(NCHUNK):
                pc = pcv.tile([128, 27], F32)
                for tap in range(9):
                    dy, dx = divmod(tap, 3)
                    for blk in range(2):
                        qo = (2 * t + dy) * W
                        lhsT = qs[(dx, blk)][:, qo: qo + 128]
                        co = (tap * 2 + blk) * 27
                        nc.tensor.matmul(
                            pc[:], lhsT=lhsT, rhs=wct[:, co: co + 27],
                            start=(tap == 0 and blk == 0),
                            stop=(tap == 8 and blk == 1),
                        )
                nc.scalar.copy(pcall[:, t * 27: (t + 1) * 27], pc[:])

        # ---- stage B: coords / weights / indices, batched over chunks ----
        wk = wkpool.tile([128, TN * 14], F32, tag="wk")

        def s(i):
            return wk[:, TN * i: TN * (i + 1)]

        pc3 = pcall[:].rearrange("p (t j) -> p t j", j=27)

        def s3(i):
            return s(i).rearrange("p (t j) -> p t j", j=9)

        # 0:sy 1:sx 2:fy 3:fx 4:y0 5:x0 6:y0c 7:x0c 8:tmp 9:tmp2
        # 10:wyA 11:wyB(->wxA/wxB reuse) 12:mod 13:omf
        v.tensor_copy(out=s3(0), in_=pc3[:, :, 0:9])     # oy
        v.tensor_copy(out=s3(1), in_=pc3[:, :, 9:18])    # ox
        v.tensor_copy(out=s3(12), in_=pc3[:, :, 18:27])  # ml
        v.scalar_tensor_tensor(s(0), s(0), ASCALE, ybt[:], op0=mult, op1=add)
        v.scalar_tensor_tensor(s(1), s(1), ASCALE, xbt[:], op0=mult, op1=add)
        v.tensor_tensor(s(12), s(12), mbt[:], op=add)
        nc.scalar.activation(s(12), s(12), mybir.ActivationFunctionType.Sigmoid)

        flr = wkpool.tile([128, TN * 2], I32, tag="flr")
        v.tensor_copy(out=flr[:, 0:TN], in_=s(0))
        v.tensor_copy(out=flr[:, TN:2 * TN], in_=s(1))
        v.tensor_copy(out=s(4), in_=flr[:, 0:TN])
        v.tensor_copy(out=s(5), in_=flr[:, TN:2 * TN])
        v.tensor_tensor(s(2), s(4), s(0), op=isgt)
        v.tensor_tensor(s(3), s(5), s(1), op=isgt)
        v.tensor_tensor(s(4), s(4), s(2), op=sub)        # y0 = floor(sy)
        v.tensor_tensor(s(5), s(5), s(3), op=sub)        # x0 = floor(sx)
        v.tensor_tensor(s(2), s(0), s(4), op=sub)        # fy
        v.tensor_tensor(s(3), s(1), s(5), op=sub)        # fx
        v.tensor_scalar(s(6), s(4), BIAS, BIAS + 64.0, op0=opmax, op1=opmin)
        v.tensor_scalar(s(7), s(5), BIAS, BIAS + 64.0, op0=opmax, op1=opmin)

        wt4 = wkpool.tile([128, TN * 4], F32, tag="wt4")  # corner weights

        # y weights (modulator folded in): wyA -> s(10), wyB -> s(11)
        v.tensor_tensor(s(8), s(6), s(4), op=sub)          # d_y
        v.tensor_scalar(s(4), s(8), 0.0, None, op0=iseq)   # e0
        v.tensor_scalar(s(9), s(8), 1.0, None, op0=iseq)   # e1
        v.tensor_scalar(s(8), s(8), -1.0, None, op0=iseq)  # em1
        v.tensor_scalar(s(13), s(2), -1.0, 1.0, op0=mult, op1=add)  # 1-fy
        v.tensor_tensor(s(10), s(4), s(13), op=mult)
        v.tensor_tensor(s(9), s(9), s(2), op=mult)
        v.tensor_tensor(s(10), s(10), s(9), op=add)        # wyA
        v.tensor_tensor(s(11), s(4), s(2), op=mult)
        v.tensor_tensor(s(8), s(8), s(13), op=mult)
        v.tensor_tensor(s(11), s(11), s(8), op=add)        # wyB
        v.tensor_tensor(s(10), s(10), s(12), op=mult)      # wyA *= mod
        v.tensor_tensor(s(11), s(11), s(12), op=mult)      # wyB *= mod

        # x weights: wxA -> s(4), wxB -> s(9)
        v.tensor_tensor(s(8), s(7), s(5), op=sub)          # d_x
        v.tensor_scalar(s(5), s(8), 0.0, None, op0=iseq)   # e0
        v.tensor_scalar(s(9), s(8), 1.0, None, op0=iseq)   # e1
        v.tensor_scalar(s(8), s(8), -1.0, None, op0=iseq)  # em1
        v.tensor_scalar(s(13), s(3), -1.0, 1.0, op0=mult, op1=add)  # 1-fx
        v.tensor_tensor(s(4), s(5), s(13), op=mult)
        v.tensor_tensor(s(9), s(9), s(3), op=mult)
        v.tensor_tensor(s(4), s(4), s(9), op=add)          # wxA
        v.tensor_tensor(s(9), s(5), s(3), op=mult)
        v.tensor_tensor(s(8), s(8), s(13), op=mult)
        v.tensor_tensor(s(9), s(9), s(8), op=add)          # wxB

        # corner weights, cols (corner*TN + t*9 + n); corner = ry*2 + xp
        v.tensor_tensor(wt4[:, 0:TN], s(10), s(4), op=mult)
        v.tensor_tensor(wt4[:, TN:2 * TN], s(10), s(9), op=mult)
        v.tensor_tensor(wt4[:, 2 * TN:3 * TN], s(11), s(4), op=mult)
        v.tensor_tensor(wt4[:, 3 * TN:4 * TN], s(11), s(9), op=mult)

        # flat pixel index: pix0 = y0c*66 + x0c - (16*66+16); corners add
        # {0, 1, 66, 67}
        v.scalar_tensor_tensor(s(0), s(6), 66.0, s(7), op0=mult, op1=add)
        v.tensor_scalar(s(1), s(0), -(BIAS * 66.0 + BIAS), None, op0=add)
        idxf = wkpool.tile([128, NCHUNK * 36], F32, tag="idxf")
        idxf3 = idxf[:].rearrange("p (t k) -> p t k", k=36)
        s1_3 = s3(1)
        for corner, delta in enumerate((0.0, 1.0, 66.0, 67.0)):
            v.tensor_scalar(idxf3[:, :, corner * 9: corner * 9 + 9], s1_3,
                            delta, None, op0=add)
        idx32 = wkpool.tile([128, NCHUNK * 36], I32, tag="idx32")
        v.tensor_copy(out=idx32[:], in_=idxf[:])

        # ---- stage C: wrapped int16 index layout for ap_gather ----
        # idxw[p16, (t*36+k)*8 + ph] = idx(pos = 16*ph + p16, t, k),
        # replicated across the 8 partition groups.
        idxw = wkpool.tile([128, NCHUNK * 288], I16, tag="idxw")
        idx16 = idx32[:].bitcast(I16).rearrange("p (j e) -> p j e", e=2)
        idxw3 = idxw[:].rearrange("p (j e) -> p j e", e=8)
        for ph in range(8):
            nc.sync.dma_start(
                out=idxw3[0:16, :, ph],
                in_=idx16[16 * ph: 16 * (ph + 1), :, 0],
            )
        for g in range(1, 8):
            nc.sync.dma_start(idxw[16 * g: 16 * (g + 1), :], idxw[0:16, :])

        # ---- stage D: gather + project + scale-accumulate per chunk ----
        with tc.tile_pool(name="pproj", bufs=3, space="PSUM") as psm, \
                tc.tile_pool(name="pacc", bufs=2, space="PSUM") as accp:
            for t in range(NCHUNK):
                gt = gpool.tile([128, 4608 * 2], F16, tag="gt")
                nc.gpsimd.ap_gather(
                    gt[:], vvt[:], idxw[:, t * 288: (t + 1) * 288],
                    channels=128, num_elems=NPIX, d=2, num_idxs=4608,
                )
                gde = dpool.tile([128, 2 * 4608], F16, tag="gde")
                v.tensor_copy(
                    out=gde[:].rearrange("p (e j) -> p e j", j=4608),
                    in_=gt[:].rearrange("p (j e) -> p e j", e=2),
                )

                acc = accp.tile([128, 256], F32, tag="acc")
                ps = [None] * 36

                def proj(term):
                    k = term  # corner*9 + n
                    n = term % 9
                    p = psm.tile([128, 256], F32, tag="ps")
                    for e in range(2):
                        nc.tensor.matmul(
                            p[:],
                            lhsT=gde[:, e * 4608 + k * 128:
                                     e * 4608 + k * 128 + 128],
                            rhs=w2t[:, (e * N + n) * 256:
                                    (e * N + n + 1) * 256],
                            start=(e == 0), stop=(e == 1),
                        )
                    ps[term] = p

                proj(0)
                proj(1)
                for term in range(36):
                    corner, n = divmod(term, 9)
                    col = corner * TN + t * 9 + n
                    sct = spool.tile([128, 256], F16, tag="sct")
                    nc.scalar.activation(
                        sct[:], ps[term][:],
                        mybir.ActivationFunctionType.Identity,
                        scale=wt4[:, col: col + 1],
                    )
                    ps[term] = None
                    if term + 2 < 36:
                        proj(term + 2)
                    nc.tensor.matmul(
                        acc[:], lhsT=idt[:], rhs=sct[:],
                        start=(term == 0), stop=(term == 35),
                    )

                outt = opool.tile([128, 256], F16, tag="outt")
                nc.scalar.copy(outt[:], acc[:])
                nc.sync.dma_start(out_d[t * 128: (t + 1) * 128, :], outt[:])

    nc.compile()
    return nc


_CACHE = {}


def _get_programs():
    if "p" not in _CACHE:
        _CACHE["p"] = _build()
    return _CACHE["p"]


def _host_prep(query, value, w_off, b_off, w_mod, b_mod, w_out):
    query = np.asarray(query, dtype=np.float32)
    value = np.asarray(value, dtype=np.float32)
    w_off = np.asarray(w_off, dtype=np.float32)
    b_off = np.asarray(b_off, dtype=np.float32)
    w_mod = np.asarray(w_mod, dtype=np.float32)
    b_mod = np.asarray(b_mod, dtype=np.float32)
    w_out = np.asarray(w_out, dtype=np.float32)

    qp = np.zeros((B, 2, 128, Hp, Wp), np.float16)
    qp[:, :, :, PAD:PAD + H, PAD:PAD + W] = query.reshape(B, 2, 128, H, W)

    vp = np.zeros((B, C, Hp, Wp), np.float32)
    vp[:, :, PAD:PAD + H, PAD:PAD + W] = value
    # [b, pair, pix, parity] with channel c = 2*pair + parity
    vv = np.ascontiguousarray(
        vp.reshape(B, 128, 2, NPIX).transpose(0, 1, 3, 2)
    ).reshape(B, 128, NPIX * 2).astype(np.float16)

    w27 = np.concatenate([w_off, w_mod], axis=0)
    wc = np.ascontiguousarray(
        w27.reshape(27, 2, 128, 9).transpose(2, 3, 1, 0)
    ).reshape(128, 9 * 2 * 27).astype(np.float16)

    # w2[p, (e*9 + n)*256 + o] = w_out[o, 2p+e, n]
    w2 = np.ascontiguousarray(
        w_out.reshape(256, 128, 2, N).transpose(1, 2, 3, 0)
    ).reshape(128, 2 * N * 256).astype(np.float16)

    ident = np.eye(128, dtype=np.float16)

    n_ar = np.arange(N)
    pn_r = (n_ar // 3 - 1).astype(np.float32)
    pn_c = (n_ar % 3 - 1).astype(np.float32)
    p_ar = np.arange(128)
    row_in_chunk = (p_ar // W).astype(np.float32)
    col_in_chunk = (p_ar % W).astype(np.float32)
    t_ar = np.arange(NCHUNK, dtype=np.float32)

    xb = (ASCALE * (col_in_chunk[:, None, None] + pn_c[None, None, :]
                    + b_off[N:2 * N][None, None, :]) - 0.5 + BIAS)
    xb = np.broadcast_to(xb, (128, NCHUNK, N)).reshape(128, TN)
    xb = np.ascontiguousarray(xb, dtype=np.float32)
    mb = np.broadcast_to(b_mod[None, None, :], (128, NCHUNK, N))
    mb = np.ascontiguousarray(mb.reshape(128, TN), dtype=np.float32)

    in_maps = []
    for core in range(NCORES):
        b, half = divmod(core, 2)
        r0 = half * ROWS
        yb = (ASCALE * (r0 + 2.0 * t_ar[None, :, None]
                        + row_in_chunk[:, None, None] + pn_r[None, None, :]
                        + b_off[0:N][None, None, :]) - 0.5 + BIAS)
        yb = np.ascontiguousarray(yb.reshape(128, TN), dtype=np.float32)
        in_maps.append({
            "qp": np.ascontiguousarray(qp[b, :, :, r0: r0 + 34, :]).reshape(
                2, 128, 34 * Wp),
            "wc": wc,
            "vv": vv[b],
            "w2": w2,
            "ybase": yb,
            "xbase": xb,
            "mbias": mb,
            "ident": ident,
        })
    return in_maps


def kernel(**inputs):
    p = _get_programs()
    in_maps = _host_prep(**inputs)
    res = run_bass_kernel_spmd(p, in_maps, core_ids=list(range(NCORES)))

    out = np.empty((B, OUTC, H, W), np.float32)
    for core in range(NCORES):
        b, half = divmod(core, 2)
        r0 = half * ROWS
        o = res.results[core]["out"].reshape(ROWS, W, OUTC)
        out[b, :, r0: r0 + ROWS, :] = o.transpose(2, 0, 1).astype(np.float32)
    return out


# revision 6
# speedup vs baseline: 8.6730x; 1.3414x over previous
"""Trainium2 Bass kernel for DeformableCrossAttentionModule (single phase).

Math (per batch b):
  offset = conv3x3(query, w_off) + b_off            # (18, H, W); ch 0:9 = dy, 9:18 = dx
  mod    = sigmoid(conv3x3(query, w_mod) + b_mod)   # (9, H, W)
  py/px  = base grid + kernel offset + offset       # (9, H, W)
  samp   = bilinear_sample(pad(value), px, py)      # (C, H, W, 9), zeros padding
  out    = einsum('chwn,ocn->bohw', samp * mod, w_out)

Sharding: 8 cores = (batch b in 0..3) x (row-half in 0..1); each core handles
32 output rows, streamed as 16 chunks of 128 positions (2 rows).

Everything runs in ONE device program per core:
  conv3x3 on PE -> sampling coords / corner weights / pixel indices on DVE
  -> bilinear 4-corner gather of channel pairs from the padded value image
     held in SBUF via the GPSIMD ap_gather instruction (per-16-partition
     wrapped index lists, rearranged on-device with 15 small SB->SB DMAs)
  -> output projection as accumulating PE matmuls (gathered data is
     channel-major so the projection needs no transpose); the per-position
     bilinear-corner weight x modulator is applied as a per-partition scale
     on the ACT engine between the projection matmul (pos-major PSUM) and an
     identity-matmul accumulation over the 36 (corner, tap) terms.

The per-core output is (2048, 256) f16 pos-major; the host transposes to
channel-major fp32 during reassembly.
"""

import sys

for _p in ("/opt/trn_rl_repo", "/opt/pypackages"):
    if _p not in sys.path:
        sys.path.insert(0, _p)

from contextlib import ExitStack

import numpy as np

import concourse.bacc as bacc
import concourse.tile as tile
from concourse import mybir
from concourse.bass_utils import run_bass_kernel_spmd

F32 = mybir.dt.float32
F16 = mybir.dt.float16
I32 = mybir.dt.int32
I16 = mybir.dt.int16

B, C, H, W = 4, 256, 64, 64
N, PAD, OUTC = 9, 1, 256
Hp, Wp = H + 2 * PAD, W + 2 * PAD  # 66, 66
NPIX = Hp * Wp                     # 4356
NCORES = 8
ROWS = H // 2          # output rows per core = 32
NCHUNK = ROWS // 2     # 16 chunks of 128 positions (2 rows x 64 cols)
TN = NCHUNK * N        # 144 = (chunk, tap) coordinate columns
ASCALE = float(Wp) / float(Wp - 1)  # 66/65, same for y since Hp == Wp
BIAS = 16.0            # keeps coords positive so trunc == floor


def _build():
    nc = bacc.Bacc("TRN2", target_bir_lowering=False, debug=False,
                   num_devices=NCORES)

    qp_d = nc.dram_tensor("qp", (2, 128, 34 * Wp), F16,
                          kind="ExternalInput").ap()
    # sharded inputs, reassembled on-device via AllGather: each core sends
    # 1/8 of wc/w2 (all-8 groups) and half of its batch's vv (pair groups)
    wcs_d = nc.dram_tensor("wcs", (128, 61), F16, kind="ExternalInput").ap()
    vvs_d = nc.dram_tensor("vvs", (128, NPIX), F16, kind="ExternalInput").ap()
    w2s_d = nc.dram_tensor("w2s", (128, 576), F16, kind="ExternalInput").ap()
    yb_d = nc.dram_tensor("ybase", (128, TN), F32, kind="ExternalInput").ap()
    xb_d = nc.dram_tensor("xbase", (128, TN), F32, kind="ExternalInput").ap()
    mb_d = nc.dram_tensor("mbias", (128, TN), F32, kind="ExternalInput").ap()
    id_d = nc.dram_tensor("ident", (128, 128), F16, kind="ExternalInput").ap()
    out_d = nc.dram_tensor("out", (NCHUNK * 128, OUTC), F16,
                           kind="ExternalOutput").ap()

    mult = mybir.AluOpType.mult
    add = mybir.AluOpType.add
    sub = mybir.AluOpType.subtract
    opmax = mybir.AluOpType.max
    opmin = mybir.AluOpType.min
    iseq = mybir.AluOpType.is_equal
    isgt = mybir.AluOpType.is_gt
    v = nc.vector

    with tile.TileContext(nc) as tc, ExitStack() as ctx:
        cpool = ctx.enter_context(tc.tile_pool(name="const", bufs=1))
        wkpool = ctx.enter_context(tc.tile_pool(name="work", bufs=1))
        gpool = ctx.enter_context(tc.tile_pool(name="gath", bufs=2))
        dpool = ctx.enter_context(tc.tile_pool(name="deint", bufs=2))
        spool = ctx.enter_context(tc.tile_pool(name="sct", bufs=3))
        opool = ctx.enter_context(tc.tile_pool(name="ostg", bufs=2))

        # ---- AllGather the sharded constants (DRAM bounce -> collective
        # -> SBUF); issued first so they overlap with the conv stage ----
        dram = ctx.enter_context(tc.tile_pool(name="dram", bufs=1,
                                              space="DRAM"))
        vv_ib = dram.tile([128, NPIX], F16)
        vv_ob = dram.tile([2 * 128, NPIX], F16)
        w2_ib = dram.tile([128, 576], F16)
        w2_ob = dram.tile([8 * 128, 576], F16)
        wc_ib = dram.tile([128, 61], F16)
        wc_ob = dram.tile([8 * 128, 61], F16)
        nc.gpsimd.dma_start(vv_ib[:], vvs_d[:])
        nc.gpsimd.dma_start(w2_ib[:], w2s_d[:])
        nc.gpsimd.dma_start(wc_ib[:], wcs_d[:])
        bypass = mybir.AluOpType.bypass
        nc.gpsimd.collective_compute(
            "AllGather", bypass,
            replica_groups=[[0, 1], [2, 3], [4, 5], [6, 7]],
            ins=[vv_ib.opt()], outs=[vv_ob.opt()],
        )
        nc.gpsimd.collective_compute(
            "AllGather", bypass,
            replica_groups=[list(range(8))],
            ins=[w2_ib.opt()], outs=[w2_ob.opt()],
        )
        nc.gpsimd.collective_compute(
            "AllGather", bypass,
            replica_groups=[list(range(8))],
            ins=[wc_ib.opt()], outs=[wc_ob.opt()],
        )

        # ---- load constants ----
        qpt = []
        for blk in range(2):
            qt = cpool.tile([128, 34 * Wp], F16, name=f"qp{blk}",
                            tag=f"qp{blk}")
            nc.sync.dma_start(qt[:], qp_d[blk])
            qpt.append(qt)
        wct = cpool.tile([128, 9 * 2 * 27], F16, tag="wc")
        for g in range(8):
            w = 61 if g < 7 else 9 * 2 * 27 - 7 * 61
            nc.sync.dma_start(wct[:, g * 61: g * 61 + w],
                              wc_ob[g * 128: g * 128 + 128, 0:w])
        vvt = cpool.tile([128, NPIX * 2], F16, tag="vv")
        for g in range(2):
            nc.sync.dma_start(vvt[:, g * NPIX: (g + 1) * NPIX],
                              vv_ob[g * 128: (g + 1) * 128, :])
        w2t = cpool.tile([128, 2 * N * 256], F16, tag="w2")
        for g in range(8):
            nc.sync.dma_start(w2t[:, g * 576: (g + 1) * 576],
                              w2_ob[g * 128: (g + 1) * 128, :])
        ybt = cpool.tile([128, TN], F32, tag="yb")
        nc.sync.dma_start(ybt[:], yb_d[:])
        xbt = cpool.tile([128, TN], F32, tag="xb")
        nc.sync.dma_start(xbt[:], xb_d[:])
        mbt = cpool.tile([128, TN], F32, tag="mb")
        nc.sync.dma_start(mbt[:], mb_d[:])
        idt = cpool.tile([128, 128], F16, tag="id")
        nc.sync.dma_start(idt[:], id_d[:])

        # ---- shifted query copies (conv lhsT needs contiguous 128-wide
        # position windows, i.e. width-64 row layout per dx shift) ----
        qs = {}
        for dx in range(3):
            for blk in range(2):
                qt = cpool.tile([128, 34 * W], F16, name=f"qs{dx}{blk}",
                                tag=f"qs{dx}{blk}")
                src = qpt[blk][:].rearrange("p (h w) -> p h w", w=Wp)
                dst = qt[:].rearrange("p (h w) -> p h w", w=W)
                v.tensor_copy(out=dst, in_=src[:, :, dx: dx + W])
                qs[(dx, blk)] = qt

        # ---- stage A: conv3x3 for all chunks -> pcall [128, 16*27] ----
        pcall = wkpool.tile([128, NCHUNK * 27], F32, tag="pcall")
        with tc.tile_pool(name="pconv", bufs=2, space="PSUM") as pcv:
            for t in range(NCHUNK):
                pc = pcv.tile([128, 27], F32)
                for tap in range(9):
                    dy, dx = divmod(tap, 3)
                    for blk in range(2):
                        qo = (2 * t + dy) * W
                        lhsT = qs[(dx, blk)][:, qo: qo + 128]
                        co = (tap * 2 + blk) * 27
                        nc.tensor.matmul(
                            pc[:], lhsT=lhsT, rhs=wct[:, co: co + 27],
                            start=(tap == 0 and blk == 0),
                            stop=(tap == 8 and blk == 1),
                        )
                nc.scalar.copy(pcall[:, t * 27: (t + 1) * 27], pc[:])

        # ---- stage B: coords / weights / indices, batched over chunks ----
        wk = wkpool.tile([128, TN * 14], F32, tag="wk")

        def s(i):
            return wk[:, TN * i: TN * (i + 1)]

        pc3 = pcall[:].rearrange("p (t j) -> p t j", j=27)

        def s3(i):
            return s(i).rearrange("p (t j) -> p t j", j=9)

        # 0:sy 1:sx 2:fy 3:fx 4:y0 5:x0 6:y0c 7:x0c 8:tmp 9:tmp2
        # 10:wyA 11:wyB(->wxA/wxB reuse) 12:mod 13:omf
        v.tensor_copy(out=s3(0), in_=pc3[:, :, 0:9])     # oy
        v.tensor_copy(out=s3(1), in_=pc3[:, :, 9:18])    # ox
        v.tensor_copy(out=s3(12), in_=pc3[:, :, 18:27])  # ml
        v.scalar_tensor_tensor(s(0), s(0), ASCALE, ybt[:], op0=mult, op1=add)
        v.scalar_tensor_tensor(s(1), s(1), ASCALE, xbt[:], op0=mult, op1=add)
        v.tensor_tensor(s(12), s(12), mbt[:], op=add)
        nc.scalar.activation(s(12), s(12), mybir.ActivationFunctionType.Sigmoid)

        flr = wkpool.tile([128, TN * 2], I32, tag="flr")
        v.tensor_copy(out=flr[:, 0:TN], in_=s(0))
        v.tensor_copy(out=flr[:, TN:2 * TN], in_=s(1))
        v.tensor_copy(out=s(4), in_=flr[:, 0:TN])
        v.tensor_copy(out=s(5), in_=flr[:, TN:2 * TN])
        v.tensor_tensor(s(2), s(4), s(0), op=isgt)
        v.tensor_tensor(s(3), s(5), s(1), op=isgt)
        v.tensor_tensor(s(4), s(4), s(2), op=sub)        # y0 = floor(sy)
        v.tensor_tensor(s(5), s(5), s(3), op=sub)        # x0 = floor(sx)
        v.tensor_tensor(s(2), s(0), s(4), op=sub)        # fy
        v.tensor_tensor(s(3), s(1), s(5), op=sub)        # fx
        v.tensor_scalar(s(6), s(4), BIAS, BIAS + 64.0, op0=opmax, op1=opmin)
        v.tensor_scalar(s(7), s(5), BIAS, BIAS + 64.0, op0=opmax, op1=opmin)

        wt4 = wkpool.tile([128, TN * 4], F32, tag="wt4")  # corner weights

        # y weights (modulator folded in): wyA -> s(10), wyB -> s(11)
        v.tensor_tensor(s(8), s(6), s(4), op=sub)          # d_y
        v.tensor_scalar(s(4), s(8), 0.0, None, op0=iseq)   # e0
        v.tensor_scalar(s(9), s(8), 1.0, None, op0=iseq)   # e1
        v.tensor_scalar(s(8), s(8), -1.0, None, op0=iseq)  # em1
        v.tensor_scalar(s(13), s(2), -1.0, 1.0, op0=mult, op1=add)  # 1-fy
        v.tensor_tensor(s(10), s(4), s(13), op=mult)
        v.tensor_tensor(s(9), s(9), s(2), op=mult)
        v.tensor_tensor(s(10), s(10), s(9), op=add)        # wyA
        v.tensor_tensor(s(11), s(4), s(2), op=mult)
        v.tensor_tensor(s(8), s(8), s(13), op=mult)
        v.tensor_tensor(s(11), s(11), s(8), op=add)        # wyB
        v.tensor_tensor(s(10), s(10), s(12), op=mult)      # wyA *= mod
        v.tensor_tensor(s(11), s(11), s(12), op=mult)      # wyB *= mod

        # x weights: wxA -> s(4), wxB -> s(9)
        v.tensor_tensor(s(8), s(7), s(5), op=sub)          # d_x
        v.tensor_scalar(s(5), s(8), 0.0, None, op0=iseq)   # e0
        v.tensor_scalar(s(9), s(8), 1.0, None, op0=iseq)   # e1
        v.tensor_scalar(s(8), s(8), -1.0, None, op0=iseq)  # em1
        v.tensor_scalar(s(13), s(3), -1.0, 1.0, op0=mult, op1=add)  # 1-fx
        v.tensor_tensor(s(4), s(5), s(13), op=mult)
        v.tensor_tensor(s(9), s(9), s(3), op=mult)
        v.tensor_tensor(s(4), s(4), s(9), op=add)          # wxA
        v.tensor_tensor(s(9), s(5), s(3), op=mult)
        v.tensor_tensor(s(8), s(8), s(13), op=mult)
        v.tensor_tensor(s(9), s(9), s(8), op=add)          # wxB

        # corner weights, cols (corner*TN + t*9 + n); corner = ry*2 + xp
        v.tensor_tensor(wt4[:, 0:TN], s(10), s(4), op=mult)
        v.tensor_tensor(wt4[:, TN:2 * TN], s(10), s(9), op=mult)
        v.tensor_tensor(wt4[:, 2 * TN:3 * TN], s(11), s(4), op=mult)
        v.tensor_tensor(wt4[:, 3 * TN:4 * TN], s(11), s(9), op=mult)

        # flat pixel index: pix0 = y0c*66 + x0c - (16*66+16); corners add
        # {0, 1, 66, 67}
        v.scalar_tensor_tensor(s(0), s(6), 66.0, s(7), op0=mult, op1=add)
        v.tensor_scalar(s(1), s(0), -(BIAS * 66.0 + BIAS), None, op0=add)
        idxf = wkpool.tile([128, NCHUNK * 36], F32, tag="idxf")
        idxf3 = idxf[:].rearrange("p (t k) -> p t k", k=36)
        s1_3 = s3(1)
        for corner, delta in enumerate((0.0, 1.0, 66.0, 67.0)):
            v.tensor_scalar(idxf3[:, :, corner * 9: corner * 9 + 9], s1_3,
                            delta, None, op0=add)
        idx32 = wkpool.tile([128, NCHUNK * 36], I32, tag="idx32")
        v.tensor_copy(out=idx32[:], in_=idxf[:])

        # ---- stage C: wrapped int16 index layout for ap_gather ----
        # idxw[p16, (t*36+k)*8 + ph] = idx(pos = 16*ph + p16, t, k),
        # replicated across the 8 partition groups.
        idxw = wkpool.tile([128, NCHUNK * 288], I16, tag="idxw")
        idx16 = idx32[:].bitcast(I16).rearrange("p (j e) -> p j e", e=2)
        idxw3 = idxw[:].rearrange("p (j e) -> p j e", e=8)
        for ph in range(8):
            nc.sync.dma_start(
                out=idxw3[0:16, :, ph],
                in_=idx16[16 * ph: 16 * (ph + 1), :, 0],
            )
        for g in range(1, 8):
            nc.sync.dma_start(idxw[16 * g: 16 * (g + 1), :], idxw[0:16, :])

        # ---- stage D: gather + project + scale-accumulate per chunk ----
        with tc.tile_pool(name="pproj", bufs=3, space="PSUM") as psm, \
                tc.tile_pool(name="pacc", bufs=2, space="PSUM") as accp:
            for t in range(NCHUNK):
                gt = gpool.tile([128, 4608 * 2], F16, tag="gt")
                nc.gpsimd.ap_gather(
                    gt[:], vvt[:], idxw[:, t * 288: (t + 1) * 288],
                    channels=128, num_elems=NPIX, d=2, num_idxs=4608,
                )
                gde = dpool.tile([128, 2 * 4608], F16, tag="gde")
                v.tensor_copy(
                    out=gde[:].rearrange("p (e j) -> p e j", j=4608),
                    in_=gt[:].rearrange("p (j e) -> p e j", e=2),
                )

                acc = accp.tile([128, 256], F32, tag="acc")
                ps = [None] * 36

                def proj(term):
                    k = term  # corner*9 + n
                    n = term % 9
                    p = psm.tile([128, 256], F32, tag="ps")
                    for e in range(2):
                        nc.tensor.matmul(
                            p[:],
                            lhsT=gde[:, e * 4608 + k * 128:
                                     e * 4608 + k * 128 + 128],
                            rhs=w2t[:, (e * N + n) * 256:
                                    (e * N + n + 1) * 256],
                            start=(e == 0), stop=(e == 1),
                        )
                    ps[term] = p

                proj(0)
                proj(1)
                for term in range(36):
                    corner, n = divmod(term, 9)
                    col = corner * TN + t * 9 + n
                    sct = spool.tile([128, 256], F16, tag="sct")
                    nc.scalar.activation(
                        sct[:], ps[term][:],
                        mybir.ActivationFunctionType.Identity,
                        scale=wt4[:, col: col + 1],
                    )
                    ps[term] = None
                    if term + 2 < 36:
                        proj(term + 2)
                    nc.tensor.matmul(
                        acc[:], lhsT=idt[:], rhs=sct[:],
                        start=(term == 0), stop=(term == 35),
                    )

                outt = opool.tile([128, 256], F16, tag="outt")
                nc.scalar.copy(outt[:], acc[:])
                nc.sync.dma_start(out_d[t * 128: (t + 1) * 128, :], outt[:])

    nc.compile()
    return nc


_CACHE = {}


def _get_programs():
    if "p" not in _CACHE:
        _CACHE["p"] = _build()
    return _CACHE["p"]


def _host_prep(query, value, w_off, b_off, w_mod, b_mod, w_out):
    query = np.asarray(query, dtype=np.float32)
    value = np.asarray(value, dtype=np.float32)
    w_off = np.asarray(w_off, dtype=np.float32)
    b_off = np.asarray(b_off, dtype=np.float32)
    w_mod = np.asarray(w_mod, dtype=np.float32)
    b_mod = np.asarray(b_mod, dtype=np.float32)
    w_out = np.asarray(w_out, dtype=np.float32)

    qp = np.zeros((B, 2, 128, Hp, Wp), np.float16)
    qp[:, :, :, PAD:PAD + H, PAD:PAD + W] = query.reshape(B, 2, 128, H, W)

    vp = np.zeros((B, C, Hp, Wp), np.float32)
    vp[:, :, PAD:PAD + H, PAD:PAD + W] = value
    # [b, pair, pix, parity] with channel c = 2*pair + parity
    vv = np.ascontiguousarray(
        vp.reshape(B, 128, 2, NPIX).transpose(0, 1, 3, 2)
    ).reshape(B, 128, NPIX * 2).astype(np.float16)

    w27 = np.concatenate([w_off, w_mod], axis=0)
    wc = np.ascontiguousarray(
        w27.reshape(27, 2, 128, 9).transpose(2, 3, 1, 0)
    ).reshape(128, 9 * 2 * 27).astype(np.float16)
    wcp = np.zeros((128, 488), np.float16)
    wcp[:, 0:486] = wc

    # w2[p, (e*9 + n)*256 + o] = w_out[o, 2p+e, n]
    w2 = np.ascontiguousarray(
        w_out.reshape(256, 128, 2, N).transpose(1, 2, 3, 0)
    ).reshape(128, 2 * N * 256).astype(np.float16)

    ident = np.eye(128, dtype=np.float16)

    n_ar = np.arange(N)
    pn_r = (n_ar // 3 - 1).astype(np.float32)
    pn_c = (n_ar % 3 - 1).astype(np.float32)
    p_ar = np.arange(128)
    row_in_chunk = (p_ar // W).astype(np.float32)
    col_in_chunk = (p_ar % W).astype(np.float32)
    t_ar = np.arange(NCHUNK, dtype=np.float32)

    xb = (ASCALE * (col_in_chunk[:, None, None] + pn_c[None, None, :]
                    + b_off[N:2 * N][None, None, :]) - 0.5 + BIAS)
    xb = np.broadcast_to(xb, (128, NCHUNK, N)).reshape(128, TN)
    xb = np.ascontiguousarray(xb, dtype=np.float32)
    mb = np.broadcast_to(b_mod[None, None, :], (128, NCHUNK, N))
    mb = np.ascontiguousarray(mb.reshape(128, TN), dtype=np.float32)

    in_maps = []
    for core in range(NCORES):
        b, half = divmod(core, 2)
        r0 = half * ROWS
        yb = (ASCALE * (r0 + 2.0 * t_ar[None, :, None]
                        + row_in_chunk[:, None, None] + pn_r[None, None, :]
                        + b_off[0:N][None, None, :]) - 0.5 + BIAS)
        yb = np.ascontiguousarray(yb.reshape(128, TN), dtype=np.float32)
        in_maps.append({
            "qp": np.ascontiguousarray(qp[b, :, :, r0: r0 + 34, :]).reshape(
                2, 128, 34 * Wp),
            "wcs": np.ascontiguousarray(
                wcp[:, core * 61: (core + 1) * 61]),
            "vvs": np.ascontiguousarray(
                vv[b][:, half * NPIX: (half + 1) * NPIX]),
            "w2s": np.ascontiguousarray(
                w2[:, core * 576: (core + 1) * 576]),
            "ybase": yb,
            "xbase": xb,
            "mbias": mb,
            "ident": ident,
        })
    return in_maps


def kernel(**inputs):
    p = _get_programs()
    in_maps = _host_prep(**inputs)
    res = run_bass_kernel_spmd(p, in_maps, core_ids=list(range(NCORES)))

    out = np.empty((B, OUTC, H, W), np.float32)
    for core in range(NCORES):
        b, half = divmod(core, 2)
        r0 = half * ROWS
        o = res.results[core]["out"].reshape(ROWS, W, OUTC)
        out[b, :, r0: r0 + ROWS, :] = o.transpose(2, 0, 1).astype(np.float32)
    return out
